# revision 29
# baseline (speedup 1.0000x reference)
"""Trainium2 Bass kernel for a 40-layer planar-flow chain (nn_Encoder_27676769255710).

Reference computation (per layer l, sequential over 40 layers):
    u_hat_l = u_l + ((-1 + softplus(w_l.u_l)) - w_l.u_l) * w_l / (w_l.w_l)
    act_l   = tanh(X_l @ w_l + b_l)
    X_{l+1} = X_l + act_l[:, None] * u_hat_l

Algebraic reformulation (u_hat and C depend only on params -> host precompute):
    C[m, j]  = w_m . u_hat_j                       (40x40)
    Y0       = X_0 @ W^T + b                       (one big matmul)
    P        = Y0;  for l: act_l = tanh(P[:, l]);  P[:, l+1:] += act_l * C[l+1:, l]
    X_out    = X_0 + A @ U_hat                     (one big matmul)

v3 structure (baseline v2 ran 158us):
  * The 40-step recurrence is FUSED into a scalar-engine-only chain:
      a_l = tanh(a_{l-1} * C[l, l-1] + p'[l])
    via activation(scale=AP, bias=AP); the rank-1 updates for columns >= l+2
    trail on a vector/pool STT stream off the critical path. Block 0's chain
    runs while block 1 is still streaming in from HBM.
  * Engine assignment respects GPSIMD's lack of a PSUM port: pool does most
    f32->bf16 casts (SBUF->SBUF) and block-1's STT stream; vector+scalar do
    the PSUM->SBUF transpose copies; vector does the x += A@U_hat adds.
  * Issue order is hand-scheduled per engine FIFO so no queue blocks a later
    phase: block-1 copies are interleaved into rec(0)'s vector stream,
    scalar gets its copies right after chain(0), and rec(1)'s p-add is
    issued before update(0)'s adds.
  * Output DMA streams per 2048-col group as soon as its adds land, block 0
    on the sync ring (after the input), block 1 on the scalar ring.

Sharding: data-parallel on the batch axis, 2048 rows -> 8 cores x 256 rows.
Params replicated.
"""

import os
import sys
from contextlib import ExitStack

import numpy as np

for _p in ("/opt/trn_rl_repo",):
    if os.path.isdir(_p) and _p not in sys.path:
        sys.path.append(_p)

import ml_dtypes

import concourse.bacc as bacc
import concourse.bass as bass
import concourse.mybir as mybir
import concourse.tile as tile
from concourse.bass_utils import run_bass_kernel_spmd

BF16 = ml_dtypes.bfloat16

S, D, L = 2048, 16384, 40
NCORES = 8
SS = S // NCORES          # 256 rows per core
NB = SS // 128            # 2 row-blocks of 128 per core
NCHUNK = D // 128         # 128 d-chunks for the transposed X@W^T contraction
GW = 2048                 # input DMA / cast group width
NG = D // GW              # 8 groups per block
OW = 2048                 # output DMA group width

f32 = mybir.dt.float32
bf16 = mybir.dt.bfloat16
Tanh = mybir.ActivationFunctionType.Tanh

_CACHE = {}


def _build_nc():
    nc = bacc.Bacc(
        "TRN2",
        target_bir_lowering=False,
        debug=False,
        num_devices=NCORES,
    )

    x_d = nc.dram_tensor("x", [SS, D], f32, kind="ExternalInput").ap()
    wt_d = nc.dram_tensor("wt", [128, NCHUNK * L], bf16, kind="ExternalInput").ap()
    uh_d = nc.dram_tensor("uh", [L, D], bf16, kind="ExternalInput").ap()
    ct_d = nc.dram_tensor("ct", [128, L * L], f32, kind="ExternalInput").ap()
    br_d = nc.dram_tensor("br", [128, L], f32, kind="ExternalInput").ap()
    id16_d = nc.dram_tensor("id16", [128, 128], bf16, kind="ExternalInput").ap()
    y_d = nc.dram_tensor("y", [SS, D], f32, kind="ExternalOutput").ap()

    with tile.TileContext(nc) as tc, ExitStack() as ctx:
        sb = ctx.enter_context(tc.tile_pool(name="sb", bufs=1))
        xbfp = ctx.enter_context(tc.tile_pool(name="xbfp", bufs=3))
        xtp = ctx.enter_context(tc.tile_pool(name="xtp", bufs=3))
        rp = ctx.enter_context(tc.tile_pool(name="rp", bufs=1))
        utp = ctx.enter_context(tc.tile_pool(name="utp", bufs=2))
        psT = ctx.enter_context(
            tc.tile_pool(name="psT", bufs=2, space=bass.MemorySpace.PSUM)
        )
        psY = ctx.enter_context(
            tc.tile_pool(name="psY", bufs=2, space=bass.MemorySpace.PSUM)
        )
        psU = ctx.enter_context(
            tc.tile_pool(name="psU", bufs=2, space=bass.MemorySpace.PSUM)
        )

        # --- resident tensors ---
        x_sb = sb.tile([128, NB, D], f32)          # whole X shard, updated in place
        wt_sb = sb.tile([128, NCHUNK * L], bf16)   # W^T chunk-packed
        uh_sb = sb.tile([L, D], bf16)              # u_hat
        ct_sb = sb.tile([128, L * L], f32)         # C^T replicated per partition
        br_sb = sb.tile([128, L], f32)             # b replicated
        id16 = sb.tile([128, 128], bf16)
        idf32 = sb.tile([128, 128], f32)           # f32 identity for f32 transposes

        # --- params (scalar HWDGE ring) + input stream (sync HWDGE ring) ---
        # tiny tensors first: br gates the y0 PSUM prefill (and with it the
        # whole scalar copy pipeline), id16 gates the first transpose. wt is
        # needed by the first y0 matmul ~15us in; ct by chain0; uh by upd0.
        nc.scalar.dma_start(br_sb[:], br_d[:])
        nc.scalar.dma_start(id16[:], id16_d[:])
        nc.scalar.dma_start(wt_sb[:], wt_d[:])
        nc.scalar.dma_start(ct_sb[:], ct_d[:])
        nc.scalar.dma_start(uh_sb[:], uh_d[:])
        nc.vector.tensor_copy(idf32[:], id16[:])
        for b in range(NB):
            for g in range(NG):
                nc.sync.dma_start(
                    x_sb[:, b, g * GW : (g + 1) * GW],
                    x_d[b * 128 : (b + 1) * 128, g * GW : (g + 1) * GW],
                )

        sc, ve, po = nc.scalar, nc.vector, nc.gpsimd

        def cast(eng, dst, src):
            if eng is sc:
                eng.copy(dst, src)
            else:
                eng.tensor_copy(dst, src)

        y0_ps = [psY.tile([128, L], f32, tag="y0", name=f"y0_{b}") for b in range(NB)]
        # bias pre-filled into the y0 accumulators (matmuls run start=False)
        for b in range(NB):
            nc.scalar.copy(y0_ps[b][:], br_sb[:])

        def group_front(b, g, cast_eng):
            """cast + 16 PE transposes for group g; returns the 2 psT tiles."""
            xbf = xbfp.tile([128, GW], bf16, tag="xbf", name=f"xbf_{b}_{g}")
            cast(cast_eng, xbf[:], x_sb[:, b, g * GW : (g + 1) * GW])
            tps = []
            for h in range(2):
                t_ps = psT.tile([128, 1024], bf16, tag="tps", name=f"tps_{b}_{g}_{h}")
                for i in range(8):
                    nc.tensor.transpose(
                        t_ps[:, i * 128 : (i + 1) * 128],
                        xbf[:, (h * 8 + i) * 128 : (h * 8 + i + 1) * 128],
                        id16[:],
                    )
                tps.append(t_ps)
            return tps

        def group_back(b, k, t_ps, copy_eng):
            """psT->SBUF copy + the 8 y0 matmuls for half-group k."""
            xt = xtp.tile([128, 1024], bf16, tag="xt", name=f"xt_{b}_{k}")
            cast(copy_eng, xt[:], t_ps[:])
            for i in range(8):
                c = k * 8 + i
                nc.tensor.matmul(
                    y0_ps[b][:],
                    xt[:, i * 128 : (i + 1) * 128],
                    wt_sb[:, c * L : (c + 1) * L],
                    start=False,
                    stop=(c == NCHUNK - 1),
                )

        def pcopy(b):
            p_t = rp.tile([128, L], f32, tag=f"p{b}", name=f"p_{b}")
            sc.copy(p_t[:], y0_ps[b][:])
            return p_t

        def rec_chain(b, p_t, stt_engines, fill_ops=(), fill_every=2,
                      sc_fill_ops=(), sc_fill_every=5):
            """Fused scalar tanh chain; STT l runs on stt_engines[l % len].
            Pool STTs use the 2-op form (tensor_scalar mult + tensor_add) and
            need an f32 act vector. fill_ops are closures issued into the
            VECTOR queue after every `fill_every` STT issues."""
            # far-tail STT updates go to the (otherwise idle) pool engine,
            # whose tensor_scalar needs an f32 scalar operand.
            a_t = rp.tile([128, L], f32, tag=f"a{b}", name=f"a_{b}")
            tmp = rp.tile([128, L], f32, tag=f"tmp{b}", name=f"tmp_{b}")
            LOOK = 8  # vector handles cols [l+2, l+2+LOOK); pool the rest
            fill_i = 0
            sc_fill_i = 0
            for l in range(L):
                if l == 0:
                    sc.activation(a_t[:, 0:1], p_t[:, 0:1], Tanh)
                else:
                    sc.activation(
                        a_t[:, l : l + 1],
                        a_t[:, l - 1 : l],
                        Tanh,
                        bias=p_t[:, l : l + 1],
                        scale=ct_sb[:, (l - 1) * L + l : (l - 1) * L + l + 1],
                    )
                if l + 2 < L:
                    hi = min(l + 2 + LOOK, L)
                    ve.scalar_tensor_tensor(
                        out=p_t[:, l + 2 : hi],
                        in0=ct_sb[:, l * L + l + 2 : l * L + hi],
                        scalar=a_t[:, l : l + 1],
                        in1=p_t[:, l + 2 : hi],
                        op0=mybir.AluOpType.mult,
                        op1=mybir.AluOpType.add,
                    )
                    if hi < L:
                        po.tensor_scalar(
                            tmp[:, hi:],
                            ct_sb[:, l * L + hi : l * L + L],
                            a_t[:, l : l + 1],
                            None,
                            mybir.AluOpType.mult,
                        )
                        po.tensor_add(p_t[:, hi:], tmp[:, hi:], p_t[:, hi:])
                    if l % sc_fill_every == 2 and sc_fill_i < len(sc_fill_ops):
                        sc_fill_ops[sc_fill_i]()
                        sc_fill_i += 1
                    if l % fill_every == fill_every - 1 and fill_i < len(fill_ops):
                        fill_ops[fill_i]()
                        fill_i += 1
            while fill_i < len(fill_ops):
                fill_ops[fill_i]()
                fill_i += 1
            while sc_fill_i < len(sc_fill_ops):
                sc_fill_ops[sc_fill_i]()
                sc_fill_i += 1
            return a_t

        def at_transpose(b, a_t, copy_eng):
            at_ps = psY.tile([L, 128], a_t.dtype, tag="y0", name=f"at_ps_{b}")
            ident = id16 if a_t.dtype == bf16 else idf32
            nc.tensor.transpose(at_ps[:], a_t[:, 0:L], ident[:])
            at_t = rp.tile([L, 128], bf16, tag=f"at{b}", name=f"at_{b}")
            cast(copy_eng, at_t[:], at_ps[:])
            return at_t

        def update(b, at_t, add_eng, out_eng):
            for n in range(D // 1024):
                u_ps = psU.tile([128, 1024], f32, tag="ups", name=f"ups_{b}_{n}")
                for h2 in range(2):
                    nc.tensor.matmul(
                        u_ps[:, h2 * 512 : (h2 + 1) * 512],
                        at_t[:],
                        uh_sb[:, n * 1024 + h2 * 512 : n * 1024 + (h2 + 1) * 512],
                        start=True,
                        stop=True,
                    )
                add_eng.tensor_add(
                    x_sb[:, b, n * 1024 : (n + 1) * 1024],
                    u_ps[:],
                    x_sb[:, b, n * 1024 : (n + 1) * 1024],
                )
                if (n + 1) % (OW // 1024) == 0:
                    g = n // (OW // 1024)
                    out_eng.dma_start(
                        y_d[b * 128 : (b + 1) * 128, g * OW : (g + 1) * OW],
                        x_sb[:, b, g * OW : (g + 1) * OW],
                    )

        # ================= issue schedule =================
        # sc:   params | br prefill | b0 copies | chain0 (+odd b1 copies as
        #       fills) | at0-copy | p1-copy | chain1 | at1-copy | 4 psU
        #       copies for pool adds
        # ve:   b0 casts | rec0 even-STTs + fills(b1 casts + even b1 copies)
        #       | rec1 STTs(even) + upd0-add fills | upd1 adds x12
        # po:   rec0 odd STTs (2-op) | rec1 odd STTs | 4 upd1 adds (SBUF)
        # sync: in x16 | out0 x8 | out1 x8

        # --- phase 1, block 0: vector casts, scalar copies
        for g in range(NG):
            tps = group_front(0, g, ve)
            group_back(0, g * 2, tps[0], ve)
            group_back(0, g * 2 + 1, tps[1], ve)
        p0 = pcopy(0)

        # --- rec(0). ve fills: per b1 group, the cast+transposes then the
        #     even half-group copy; sc fills: the odd half-group copies
        #     (issued between chain ACTs). psT rotation order k0,k1,k2,...
        #     is preserved across the two streams.
        b1_tps = {}
        b1_xt = {}

        def mk_front(g):
            def op():
                tps = group_front(1, g, ve)
                b1_tps[g * 2] = tps[0]
                b1_tps[g * 2 + 1] = tps[1]
            return op

        def mk_copy(k, eng):
            def op():
                xt = xtp.tile([128, 1024], bf16, tag="xt", name=f"xt_1_{k}")
                cast(eng, xt[:], b1_tps.pop(k)[:])
                b1_xt[k] = xt
            return op

        def mk_mms(k):
            def op():
                xt = b1_xt.pop(k)
                for i in range(8):
                    c = k * 8 + i
                    nc.tensor.matmul(
                        y0_ps[1][:],
                        xt[:, i * 128 : (i + 1) * 128],
                        wt_sb[:, c * L : (c + 1) * L],
                        start=False,
                        stop=(c == NCHUNK - 1),
                    )
            return op

        # ve fills carry fronts, even copies, and ALL y0 matmuls in strict
        # k order (PE FIFO must see transposes+matmuls in rotation order);
        # sc fills carry only the odd copies.
        fill_ops = []
        for g in range(NG):
            fill_ops.append(mk_front(g))
            fill_ops.append(mk_copy(g * 2, ve))
            fill_ops.append(mk_mms(g * 2))
            fill_ops.append(mk_mms(g * 2 + 1))
        sc_fill_ops = [mk_copy(g * 2 + 1, sc) for g in range(NG)]
        a0 = rec_chain(0, p0, [ve], fill_ops=fill_ops, fill_every=1,
                       sc_fill_ops=sc_fill_ops, sc_fill_every=4)

        # --- at(0), p-copy(1), update(0) PE matmuls
        at0 = at_transpose(0, a0, sc)
        p1 = pcopy(1)

        u_ps0 = []
        for n in range(D // 1024):
            u_ps = psU.tile([128, 1024], f32, tag="ups", name=f"ups_0_{n}")
            for h2 in range(2):
                nc.tensor.matmul(
                    u_ps[:, h2 * 512 : (h2 + 1) * 512],
                    at0[:],
                    uh_sb[:, n * 1024 + h2 * 512 : n * 1024 + (h2 + 1) * 512],
                    start=True,
                    stop=True,
                )
            u_ps0.append(u_ps)

        # --- rec(1): scalar chain; STTs split ve/po; update(0)'s 16 adds
        #     ride the vector stream as fills, out-DMA on sync per group.
        def mk_add0(n):
            def op():
                ve.tensor_add(
                    x_sb[:, 0, n * 1024 : (n + 1) * 1024],
                    u_ps0[n][:],
                    x_sb[:, 0, n * 1024 : (n + 1) * 1024],
                )
                nc.sync.dma_start(
                    y_d[0:128, n * 1024 : (n + 1) * 1024],
                    x_sb[:, 0, n * 1024 : (n + 1) * 1024],
                )
            return op

        a1 = rec_chain(
            1, p1, [ve], fill_ops=[mk_add0(n) for n in range(16)], fill_every=1
        )
        at1 = at_transpose(1, a1, sc)

        # --- update(1): PE mms; adds: vector x12, pool x4 (via a scalar
        #     PSUM->SBUF copy since GPSIMD has no PSUM port); out on sync.
        PO_TILES = (1, 5, 9, 13)
        for n in range(D // 1024):
            u_ps = psU.tile([128, 1024], f32, tag="ups", name=f"ups_1_{n}")
            for h2 in range(2):
                nc.tensor.matmul(
                    u_ps[:, h2 * 512 : (h2 + 1) * 512],
                    at1[:],
                    uh_sb[:, n * 1024 + h2 * 512 : n * 1024 + (h2 + 1) * 512],
                    start=True,
                    stop=True,
                )
            if n in PO_TILES:
                utmp = utp.tile([128, 1024], f32, tag="utmp", name=f"utmp_{n}")
                sc.copy(utmp[:], u_ps[:])
                po.tensor_add(
                    x_sb[:, 1, n * 1024 : (n + 1) * 1024],
                    utmp[:],
                    x_sb[:, 1, n * 1024 : (n + 1) * 1024],
                )
            else:
                ve.tensor_add(
                    x_sb[:, 1, n * 1024 : (n + 1) * 1024],
                    u_ps[:],
                    x_sb[:, 1, n * 1024 : (n + 1) * 1024],
                )
            nc.sync.dma_start(
                y_d[128:256, n * 1024 : (n + 1) * 1024],
                x_sb[:, 1, n * 1024 : (n + 1) * 1024],
            )

    nc.compile()
    return nc


def _prep_params(ws: np.ndarray, us: np.ndarray, bs: np.ndarray) -> dict:
    """Host-side precompute of the tiny flow-parameter tensors (f64 for accuracy)."""
    w = ws.astype(np.float64)
    u = us.astype(np.float64)
    wu = np.sum(w * u, axis=1)
    ww = np.sum(w * w, axis=1)
    m = -1.0 + np.logaddexp(0.0, wu)  # softplus
    u_hat = u + ((m - wu) / ww)[:, None] * w              # [L, D]
    C = w @ u_hat.T                                        # C[m, j] = w_m . u_hat_j

    # W^T packed for the chunked contraction: wt[p, c*L + l] = W[l, c*128 + p]
    wt = np.ascontiguousarray(
        ws.astype(np.float32).T.reshape(NCHUNK, 128, L).transpose(1, 0, 2)
    ).reshape(128, NCHUNK * L)

    # C^T replicated per partition: ct[p, j*L + m] = C[m, j]
    ct = np.tile(np.ascontiguousarray(C.T.astype(np.float32)).reshape(1, L * L), (128, 1))
    br = np.tile(bs.astype(np.float32).reshape(1, L), (128, 1))

    return {
        "wt": wt.astype(BF16),
        "uh": u_hat.astype(np.float32).astype(BF16),
        "ct": np.ascontiguousarray(ct, dtype=np.float32),
        "br": np.ascontiguousarray(br, dtype=np.float32),
        "id16": np.eye(128, dtype=np.float32).astype(BF16),
    }


def run(X, ws, us, bs, trace=False, **trace_kwargs):
    if "nc" not in _CACHE:
        _CACHE["nc"] = _build_nc()
    nc = _CACHE["nc"]

    params = _prep_params(np.asarray(ws), np.asarray(us), np.asarray(bs))
    X = np.ascontiguousarray(np.asarray(X, dtype=np.float32))
    in_maps = [
        {"x": X[c * SS : (c + 1) * SS], **params} for c in range(NCORES)
    ]
    res = run_bass_kernel_spmd(
        nc, in_maps, list(range(NCORES)), trace=trace, **trace_kwargs
    )
    out = np.concatenate([res.results[c]["y"] for c in range(NCORES)], axis=0)
    return out, res


def kernel(X, ws, us, bs):
    out, _ = run(X, ws, us, bs, trace=False)
    return out


# revision 31
# speedup vs baseline: 1.1889x; 1.1889x over previous
"""Trainium2 Bass kernel for a 40-layer planar-flow chain (nn_Encoder_27676769255710).

Reference computation (per layer l, sequential over 40 layers):
    u_hat_l = u_l + ((-1 + softplus(w_l.u_l)) - w_l.u_l) * w_l / (w_l.w_l)
    act_l   = tanh(X_l @ w_l + b_l)
    X_{l+1} = X_l + act_l[:, None] * u_hat_l

Algebraic reformulation (u_hat and C depend only on params -> host precompute):
    C[m, j]  = w_m . u_hat_j                       (40x40)
    Y0       = X_0 @ W^T + b                       (one big matmul)
    P        = Y0;  for l: act_l = tanh(P[:, l]);  P[:, l+1:] += act_l * C[l+1:, l]
    X_out    = X_0 + A @ U_hat                     (one big matmul)

v3 structure (baseline v2 ran 158us):
  * The 40-step recurrence is FUSED into a scalar-engine-only chain:
      a_l = tanh(a_{l-1} * C[l, l-1] + p'[l])
    via activation(scale=AP, bias=AP); the rank-1 updates for columns >= l+2
    trail on a vector/pool STT stream off the critical path. Block 0's chain
    runs while block 1 is still streaming in from HBM.
  * Engine assignment respects GPSIMD's lack of a PSUM port: pool does most
    f32->bf16 casts (SBUF->SBUF) and block-1's STT stream; vector+scalar do
    the PSUM->SBUF transpose copies; vector does the x += A@U_hat adds.
  * Issue order is hand-scheduled per engine FIFO so no queue blocks a later
    phase: block-1 copies are interleaved into rec(0)'s vector stream,
    scalar gets its copies right after chain(0), and rec(1)'s p-add is
    issued before update(0)'s adds.
  * Output DMA streams per 2048-col group as soon as its adds land, block 0
    on the sync ring (after the input), block 1 on the scalar ring.

Sharding: data-parallel on the batch axis, 2048 rows -> 8 cores x 256 rows.
Params replicated.
"""

import os
import sys
from contextlib import ExitStack

import numpy as np

for _p in ("/opt/trn_rl_repo",):
    if os.path.isdir(_p) and _p not in sys.path:
        sys.path.append(_p)

import ml_dtypes

import concourse.bacc as bacc
import concourse.bass as bass
import concourse.mybir as mybir
import concourse.tile as tile
from concourse.bass_utils import run_bass_kernel_spmd

BF16 = ml_dtypes.bfloat16

S, D, L = 2048, 16384, 40
NCORES = 8
SS = S // NCORES          # 256 rows per core
NB = SS // 128            # 2 row-blocks of 128 per core
NCHUNK = D // 128         # 128 d-chunks for the transposed X@W^T contraction
GW = 2048                 # input DMA / cast group width
NG = D // GW              # 8 groups per block
OW = 2048                 # output DMA group width

f32 = mybir.dt.float32
bf16 = mybir.dt.bfloat16
Tanh = mybir.ActivationFunctionType.Tanh

_CACHE = {}


def _build_nc():
    nc = bacc.Bacc(
        "TRN2",
        target_bir_lowering=False,
        debug=False,
        num_devices=NCORES,
    )

    x_d = nc.dram_tensor("x", [SS, D], f32, kind="ExternalInput").ap()
    wt_d = nc.dram_tensor("wt", [128, NCHUNK * L], bf16, kind="ExternalInput").ap()
    uh_d = nc.dram_tensor("uh", [L, D], bf16, kind="ExternalInput").ap()
    ct_d = nc.dram_tensor("ct", [128, L * L], f32, kind="ExternalInput").ap()
    br_d = nc.dram_tensor("br", [128, L], f32, kind="ExternalInput").ap()
    id16_d = nc.dram_tensor("id16", [128, 128], bf16, kind="ExternalInput").ap()
    y_d = nc.dram_tensor("y", [SS, D], f32, kind="ExternalOutput").ap()

    with tile.TileContext(nc) as tc, ExitStack() as ctx:
        sb = ctx.enter_context(tc.tile_pool(name="sb", bufs=1))
        xbfp = ctx.enter_context(tc.tile_pool(name="xbfp", bufs=3))
        xtp = ctx.enter_context(tc.tile_pool(name="xtp", bufs=3))
        rp = ctx.enter_context(tc.tile_pool(name="rp", bufs=1))
        utp = ctx.enter_context(tc.tile_pool(name="utp", bufs=2))
        psT = ctx.enter_context(
            tc.tile_pool(name="psT", bufs=2, space=bass.MemorySpace.PSUM)
        )
        psY = ctx.enter_context(
            tc.tile_pool(name="psY", bufs=2, space=bass.MemorySpace.PSUM)
        )
        psU = ctx.enter_context(
            tc.tile_pool(name="psU", bufs=2, space=bass.MemorySpace.PSUM)
        )

        # --- resident tensors ---
        x_sb = sb.tile([128, NB, D], f32)          # whole X shard, updated in place
        wt_sb = sb.tile([128, NCHUNK * L], bf16)   # W^T chunk-packed
        uh_sb = sb.tile([L, D], bf16)              # u_hat
        ct_sb = sb.tile([128, L * L], f32)         # C^T replicated per partition
        br_sb = sb.tile([128, L], f32)             # b replicated
        id16 = sb.tile([128, 128], bf16)
        idf32 = sb.tile([128, 128], f32)           # f32 identity for f32 transposes

        # --- params (scalar HWDGE ring) + input stream (sync HWDGE ring) ---
        # tiny tensors first: br gates the y0 PSUM prefill (and with it the
        # whole scalar copy pipeline), id16 gates the first transpose. wt is
        # needed by the first y0 matmul ~15us in; ct by chain0; uh by upd0.
        nc.scalar.dma_start(br_sb[:], br_d[:])
        nc.scalar.dma_start(id16[:], id16_d[:])
        nc.scalar.dma_start(wt_sb[:], wt_d[:])
        nc.scalar.dma_start(ct_sb[:], ct_d[:])
        nc.scalar.dma_start(uh_sb[:], uh_d[:])
        nc.vector.tensor_copy(idf32[:], id16[:])
        # stripe the X input across BOTH HWDGE rings: a single queue leaves
        # the 16 DMA engines at ~55% duty; two queues issue concurrently.
        for b in range(NB):
            for g in range(NG):
                eng = nc.sync if g % 2 == 0 else nc.scalar
                eng.dma_start(
                    x_sb[:, b, g * GW : (g + 1) * GW],
                    x_d[b * 128 : (b + 1) * 128, g * GW : (g + 1) * GW],
                )

        sc, ve, po = nc.scalar, nc.vector, nc.gpsimd

        def cast(eng, dst, src):
            if eng is sc:
                eng.copy(dst, src)
            else:
                eng.tensor_copy(dst, src)

        y0_ps = [psY.tile([128, L], f32, tag="y0", name=f"y0_{b}") for b in range(NB)]
        # bias pre-filled into the y0 accumulators (matmuls run start=False)
        for b in range(NB):
            nc.scalar.copy(y0_ps[b][:], br_sb[:])

        def group_front(b, g, cast_eng):
            """cast + 16 PE transposes for group g; returns the 2 psT tiles."""
            xbf = xbfp.tile([128, GW], bf16, tag="xbf", name=f"xbf_{b}_{g}")
            cast(cast_eng, xbf[:], x_sb[:, b, g * GW : (g + 1) * GW])
            tps = []
            for h in range(2):
                t_ps = psT.tile([128, 1024], bf16, tag="tps", name=f"tps_{b}_{g}_{h}")
                for i in range(8):
                    nc.tensor.transpose(
                        t_ps[:, i * 128 : (i + 1) * 128],
                        xbf[:, (h * 8 + i) * 128 : (h * 8 + i + 1) * 128],
                        id16[:],
                    )
                tps.append(t_ps)
            return tps

        def group_back(b, k, t_ps, copy_eng):
            """psT->SBUF copy + the 8 y0 matmuls for half-group k."""
            xt = xtp.tile([128, 1024], bf16, tag="xt", name=f"xt_{b}_{k}")
            cast(copy_eng, xt[:], t_ps[:])
            for i in range(8):
                c = k * 8 + i
                nc.tensor.matmul(
                    y0_ps[b][:],
                    xt[:, i * 128 : (i + 1) * 128],
                    wt_sb[:, c * L : (c + 1) * L],
                    start=False,
                    stop=(c == NCHUNK - 1),
                )

        def pcopy(b):
            p_t = rp.tile([128, L], f32, tag=f"p{b}", name=f"p_{b}")
            sc.copy(p_t[:], y0_ps[b][:])
            return p_t

        def rec_chain(b, p_t, stt_engines, fill_ops=(), fill_every=2,
                      sc_fill_ops=(), sc_fill_every=5):
            """Fused scalar tanh chain; STT l runs on stt_engines[l % len].
            Pool STTs use the 2-op form (tensor_scalar mult + tensor_add) and
            need an f32 act vector. fill_ops are closures issued into the
            VECTOR queue after every `fill_every` STT issues."""
            use_po = any(e is po for e in stt_engines)
            a_t = rp.tile(
                [128, L], f32 if use_po else bf16, tag=f"a{b}", name=f"a_{b}"
            )
            tmp = None
            if use_po:
                tmp = rp.tile([128, L], f32, tag=f"tmp{b}", name=f"tmp_{b}")
            fill_i = 0
            sc_fill_i = 0
            for l in range(L):
                if l == 0:
                    sc.activation(a_t[:, 0:1], p_t[:, 0:1], Tanh)
                else:
                    sc.activation(
                        a_t[:, l : l + 1],
                        a_t[:, l - 1 : l],
                        Tanh,
                        bias=p_t[:, l : l + 1],
                        scale=ct_sb[:, (l - 1) * L + l : (l - 1) * L + l + 1],
                    )
                if l + 2 < L:
                    eng = stt_engines[l % len(stt_engines)]
                    if eng is po:
                        eng.tensor_scalar(
                            tmp[:, l + 2 :],
                            ct_sb[:, l * L + l + 2 : l * L + L],
                            a_t[:, l : l + 1],
                            None,
                            mybir.AluOpType.mult,
                        )
                        eng.tensor_add(
                            p_t[:, l + 2 :], tmp[:, l + 2 :], p_t[:, l + 2 :]
                        )
                    else:
                        eng.scalar_tensor_tensor(
                            out=p_t[:, l + 2 :],
                            in0=ct_sb[:, l * L + l + 2 : l * L + L],
                            scalar=a_t[:, l : l + 1],
                            in1=p_t[:, l + 2 :],
                            op0=mybir.AluOpType.mult,
                            op1=mybir.AluOpType.add,
                        )
                    if l % sc_fill_every == 2 and sc_fill_i < len(sc_fill_ops):
                        sc_fill_ops[sc_fill_i]()
                        sc_fill_i += 1
                    if l % fill_every == fill_every - 1 and fill_i < len(fill_ops):
                        fill_ops[fill_i]()
                        fill_i += 1
            while fill_i < len(fill_ops):
                fill_ops[fill_i]()
                fill_i += 1
            while sc_fill_i < len(sc_fill_ops):
                sc_fill_ops[sc_fill_i]()
                sc_fill_i += 1
            return a_t

        def at_transpose(b, a_t, copy_eng):
            at_ps = psY.tile([L, 128], a_t.dtype, tag="y0", name=f"at_ps_{b}")
            ident = id16 if a_t.dtype == bf16 else idf32
            nc.tensor.transpose(at_ps[:], a_t[:, 0:L], ident[:])
            at_t = rp.tile([L, 128], bf16, tag=f"at{b}", name=f"at_{b}")
            cast(copy_eng, at_t[:], at_ps[:])
            return at_t

        def update(b, at_t, add_eng, out_eng):
            for n in range(D // 1024):
                u_ps = psU.tile([128, 1024], f32, tag="ups", name=f"ups_{b}_{n}")
                for h2 in range(2):
                    nc.tensor.matmul(
                        u_ps[:, h2 * 512 : (h2 + 1) * 512],
                        at_t[:],
                        uh_sb[:, n * 1024 + h2 * 512 : n * 1024 + (h2 + 1) * 512],
                        start=True,
                        stop=True,
                    )
                add_eng.tensor_add(
                    x_sb[:, b, n * 1024 : (n + 1) * 1024],
                    u_ps[:],
                    x_sb[:, b, n * 1024 : (n + 1) * 1024],
                )
                if (n + 1) % (OW // 1024) == 0:
                    g = n // (OW // 1024)
                    out_eng.dma_start(
                        y_d[b * 128 : (b + 1) * 128, g * OW : (g + 1) * OW],
                        x_sb[:, b, g * OW : (g + 1) * OW],
                    )

        # ================= issue schedule =================
        # sc:   params | br prefill | b0 copies | chain0 (+odd b1 copies as
        #       fills) | at0-copy | p1-copy | chain1 | at1-copy | 4 psU
        #       copies for pool adds
        # ve:   b0 casts | rec0 even-STTs + fills(b1 casts + even b1 copies)
        #       | rec1 STTs(even) + upd0-add fills | upd1 adds x12
        # po:   rec0 odd STTs (2-op) | rec1 odd STTs | 4 upd1 adds (SBUF)
        # sync: in x16 | out0 x8 | out1 x8

        # --- phase 1, block 0: vector casts, scalar copies
        for g in range(NG):
            tps = group_front(0, g, ve)
            group_back(0, g * 2, tps[0], ve)
            group_back(0, g * 2 + 1, tps[1], ve)
        p0 = pcopy(0)

        # --- rec(0). ve fills: per b1 group, the cast+transposes then the
        #     even half-group copy; sc fills: the odd half-group copies
        #     (issued between chain ACTs). psT rotation order k0,k1,k2,...
        #     is preserved across the two streams.
        b1_tps = {}
        b1_xt = {}

        def mk_front(g):
            def op():
                tps = group_front(1, g, ve)
                b1_tps[g * 2] = tps[0]
                b1_tps[g * 2 + 1] = tps[1]
            return op

        def mk_copy(k, eng):
            def op():
                xt = xtp.tile([128, 1024], bf16, tag="xt", name=f"xt_1_{k}")
                cast(eng, xt[:], b1_tps.pop(k)[:])
                b1_xt[k] = xt
            return op

        def mk_mms(k):
            def op():
                xt = b1_xt.pop(k)
                for i in range(8):
                    c = k * 8 + i
                    nc.tensor.matmul(
                        y0_ps[1][:],
                        xt[:, i * 128 : (i + 1) * 128],
                        wt_sb[:, c * L : (c + 1) * L],
                        start=False,
                        stop=(c == NCHUNK - 1),
                    )
            return op

        # ve fills carry fronts, even copies, and ALL y0 matmuls in strict
        # k order (PE FIFO must see transposes+matmuls in rotation order);
        # sc fills carry only the odd copies.
        fill_ops = []
        for g in range(NG):
            fill_ops.append(mk_front(g))
            fill_ops.append(mk_copy(g * 2, ve))
            fill_ops.append(mk_mms(g * 2))
            fill_ops.append(mk_mms(g * 2 + 1))
        sc_fill_ops = [mk_copy(g * 2 + 1, sc) for g in range(NG)]
        a0 = rec_chain(0, p0, [ve], fill_ops=fill_ops, fill_every=1,
                       sc_fill_ops=sc_fill_ops, sc_fill_every=4)

        # --- at(0), p-copy(1), update(0) PE matmuls
        at0 = at_transpose(0, a0, sc)
        p1 = pcopy(1)

        u_ps0 = []
        for n in range(D // 1024):
            u_ps = psU.tile([128, 1024], f32, tag="ups", name=f"ups_0_{n}")
            for h2 in range(2):
                nc.tensor.matmul(
                    u_ps[:, h2 * 512 : (h2 + 1) * 512],
                    at0[:],
                    uh_sb[:, n * 1024 + h2 * 512 : n * 1024 + (h2 + 1) * 512],
                    start=True,
                    stop=True,
                )
            u_ps0.append(u_ps)

        # --- rec(1): scalar chain; STTs split ve/po; update(0)'s 16 adds
        #     ride the vector stream as fills, out-DMA on sync per group.
        def mk_add0(n):
            def op():
                ve.tensor_add(
                    x_sb[:, 0, n * 1024 : (n + 1) * 1024],
                    u_ps0[n][:],
                    x_sb[:, 0, n * 1024 : (n + 1) * 1024],
                )
                nc.sync.dma_start(
                    y_d[0:128, n * 1024 : (n + 1) * 1024],
                    x_sb[:, 0, n * 1024 : (n + 1) * 1024],
                )
            return op

        a1 = rec_chain(
            1, p1, [ve], fill_ops=[mk_add0(n) for n in range(16)], fill_every=1
        )
        at1 = at_transpose(1, a1, sc)

        # --- update(1): PE mms; adds: vector x12, pool x4 (via a scalar
        #     PSUM->SBUF copy since GPSIMD has no PSUM port); out on sync.
        PO_TILES = (1, 5, 9, 13)
        for n in range(D // 1024):
            u_ps = psU.tile([128, 1024], f32, tag="ups", name=f"ups_1_{n}")
            for h2 in range(2):
                nc.tensor.matmul(
                    u_ps[:, h2 * 512 : (h2 + 1) * 512],
                    at1[:],
                    uh_sb[:, n * 1024 + h2 * 512 : n * 1024 + (h2 + 1) * 512],
                    start=True,
                    stop=True,
                )
            if n in PO_TILES:
                utmp = utp.tile([128, 1024], f32, tag="utmp", name=f"utmp_{n}")
                sc.copy(utmp[:], u_ps[:])
                po.tensor_add(
                    x_sb[:, 1, n * 1024 : (n + 1) * 1024],
                    utmp[:],
                    x_sb[:, 1, n * 1024 : (n + 1) * 1024],
                )
            else:
                ve.tensor_add(
                    x_sb[:, 1, n * 1024 : (n + 1) * 1024],
                    u_ps[:],
                    x_sb[:, 1, n * 1024 : (n + 1) * 1024],
                )
            nc.sync.dma_start(
                y_d[128:256, n * 1024 : (n + 1) * 1024],
                x_sb[:, 1, n * 1024 : (n + 1) * 1024],
            )

    nc.compile()
    return nc


def _prep_params(ws: np.ndarray, us: np.ndarray, bs: np.ndarray) -> dict:
    """Host-side precompute of the tiny flow-parameter tensors (f64 for accuracy)."""
    w = ws.astype(np.float64)
    u = us.astype(np.float64)
    wu = np.sum(w * u, axis=1)
    ww = np.sum(w * w, axis=1)
    m = -1.0 + np.logaddexp(0.0, wu)  # softplus
    u_hat = u + ((m - wu) / ww)[:, None] * w              # [L, D]
    C = w @ u_hat.T                                        # C[m, j] = w_m . u_hat_j

    # W^T packed for the chunked contraction: wt[p, c*L + l] = W[l, c*128 + p]
    wt = np.ascontiguousarray(
        ws.astype(np.float32).T.reshape(NCHUNK, 128, L).transpose(1, 0, 2)
    ).reshape(128, NCHUNK * L)

    # C^T replicated per partition: ct[p, j*L + m] = C[m, j]
    ct = np.tile(np.ascontiguousarray(C.T.astype(np.float32)).reshape(1, L * L), (128, 1))
    br = np.tile(bs.astype(np.float32).reshape(1, L), (128, 1))

    return {
        "wt": wt.astype(BF16),
        "uh": u_hat.astype(np.float32).astype(BF16),
        "ct": np.ascontiguousarray(ct, dtype=np.float32),
        "br": np.ascontiguousarray(br, dtype=np.float32),
        "id16": np.eye(128, dtype=np.float32).astype(BF16),
    }


def run(X, ws, us, bs, trace=False, **trace_kwargs):
    if "nc" not in _CACHE:
        _CACHE["nc"] = _build_nc()
    nc = _CACHE["nc"]

    params = _prep_params(np.asarray(ws), np.asarray(us), np.asarray(bs))
    X = np.ascontiguousarray(np.asarray(X, dtype=np.float32))
    in_maps = [
        {"x": X[c * SS : (c + 1) * SS], **params} for c in range(NCORES)
    ]
    res = run_bass_kernel_spmd(
        nc, in_maps, list(range(NCORES)), trace=trace, **trace_kwargs
    )
    out = np.concatenate([res.results[c]["y"] for c in range(NCORES)], axis=0)
    return out, res


def kernel(X, ws, us, bs):
    out, _ = run(X, ws, us, bs, trace=False)
    return out
